# revision 34
# baseline (speedup 1.0000x reference)
"""Fused multi-head attention block (qkv + rmsnorm + rope + sdpa + proj) for
Trainium2, sharded over 8 NeuronCores as batch x head-half (Megatron-style).

Shapes (hardcoded): B=4, N=2048, C=1024, H=16, D=64.
Each core handles one batch and 8 heads (= 512 feature columns).
Host folds qn_w/kn_w into rope tables, sums the two per-batch partial
projection outputs and transposes back to [B, N, C].
"""
import os
import sys

os.environ.setdefault("NEURON_RT_RESET_CORES", "1")
sys.path.insert(0, "/opt/trn_rl_repo")

import ml_dtypes
import numpy as np

import concourse.bass as bass
import concourse.mybir as mybir
import concourse.tile as tile
from concourse import bacc
from concourse.bass_utils import run_bass_kernel_spmd
from concourse.masks import make_identity

dt = mybir.dt
F32 = dt.float32
F32R = dt.float32r
BF16 = dt.bfloat16
AF = mybir.ActivationFunctionType

B, N, C, H, D = 4, 2048, 1024, 16, 64
HL = H // 2          # heads per core = 8
FL = HL * D          # local features = 512
EPS = 1e-6
SCALE = 1.0 / np.sqrt(D)
NCHUNK = N // 128    # 16
KC = C // 128        # 8  (c_in chunks)
IH = 2               # i-halves of 1024 in phase 2
# Schraudolph offload: exp units (2*jc+half) % SCHRAUD == 1 go to DVE.
# 0 disables; 4 = 25% of units, 3 = 33%, 2 = 50%.
SCHRAUD = int(os.environ.get("KERNEL_SCHRAUD", "0"))
SCHRAUD_C = float(os.environ.get("KERNEL_SCHRAUD_C", "0.0434"))
S_A = 128.0 * 1.4426950408889634 * (1.0 / np.sqrt(64))
S_B = 128.0 * (127.0 - SCHRAUD_C)

_PROGRAM = None


def _build_program(with_qkv_bias, with_proj_bias, bench_reps=0, qkv_bf16=True):
    XDT = BF16 if qkv_bf16 else F32R
    xlb, nqb, scb, ptb, avb = (5, 6, 10, 4, 2) if qkv_bf16 else (3, 4, 8, 3, 1)
    nc = bacc.Bacc("TRN2", target_bir_lowering=False, debug=False, num_devices=8)

    i_xT = nc.dram_tensor("xT", [NCHUNK, 128, KC, 128], XDT, kind="ExternalInput")
    i_wq = nc.dram_tensor("wq", [C, FL], XDT, kind="ExternalInput")
    i_wk = nc.dram_tensor("wk", [C, FL], XDT, kind="ExternalInput")
    i_wv = nc.dram_tensor("wv", [C, FL], XDT, kind="ExternalInput")
    i_wp = nc.dram_tensor("wp", [FL, C], BF16, kind="ExternalInput")
    if with_qkv_bias:
        i_qkvb = nc.dram_tensor("qkvb", [1, 3 * FL], XDT, kind="ExternalInput")
        i_ones1 = nc.dram_tensor("ones1", [1, 128], XDT, kind="ExternalInput")
    i_raq = nc.dram_tensor("raq", [N, D], BF16, kind="ExternalInput")
    i_rbq = nc.dram_tensor("rbq", [N, D], BF16, kind="ExternalInput")
    i_rak = nc.dram_tensor("rak", [N, D], BF16, kind="ExternalInput")
    i_rbk = nc.dram_tensor("rbk", [N, D], BF16, kind="ExternalInput")
    if with_proj_bias:
        i_pb = nc.dram_tensor("pb", [128, KC], F32, kind="ExternalInput")
    o_FT = nc.dram_tensor("FT", [C, N], F32, kind="ExternalOutput")

    from contextlib import ExitStack
    with tile.TileContext(nc) as tc:
        with ExitStack() as ctx:
            pp = ctx.enter_context(tc.tile_pool(name="persist", bufs=1))
            wpool = ctx.enter_context(tc.tile_pool(name="wpool", bufs=1))
            xload = ctx.enter_context(tc.tile_pool(name="xload", bufs=xlb))
            scratch = ctx.enter_context(tc.tile_pool(name="scratch", bufs=scb))
            natq = ctx.enter_context(tc.tile_pool(name="natq", bufs=nqb))
            natk = ctx.enter_context(tc.tile_pool(name="natk", bufs=nqb))
            sumsp = ctx.enter_context(tc.tile_pool(name="sumsp", bufs=4))
            ptp = ctx.enter_context(tc.tile_pool(name="ptp", bufs=ptb))
            rlp = ctx.enter_context(tc.tile_pool(name="rlp", bufs=2))
            rbp = ctx.enter_context(tc.tile_pool(name="rbp", bufs=2))
            outp = ctx.enter_context(tc.tile_pool(name="outp", bufs=4))
            # one PSUM pool, 4 tags x 2 banks (8 banks total), shared by all
            # phases: scA/scB + avA/avB
            psp = ctx.enter_context(tc.tile_pool(name="psp", bufs=1, space="PSUM"))

            # ---- persistent tensors ----
            # DMA emission order matters: the sync queue drains in order, so
            # interleave the first x chunks with the qkv weights (q first)
            # and push wp (phase-3-only) to the back.
            xm_t = {}

            def xm_prefetch(m):
                if m >= NCHUNK:
                    return
                xm = xload.tile([128, KC, 128], XDT, tag="xm", name=f"xm{m}")
                nc.sync.dma_start(out=xm[:], in_=i_xT[m])
                xm_t[m] = xm

            xm_prefetch(0)
            w_sb = {}
            # per-kc weight chunks: the m=0 qkv matmuls accumulate kc=0..7
            # in order, so they can chase the chunk DMAs instead of waiting
            # for the full 1MB tile
            for pf, (nm, src) in enumerate(
                    (("q", i_wq), ("k", i_wk), ("v", i_wv))):
                t = wpool.tile([128, KC, FL], XDT, tag=f"w{nm}")
                srcv = src[:].rearrange("(kc c) f -> c kc f", c=128)
                for kc in range(KC):
                    nc.sync.dma_start(out=t[:, kc, :], in_=srcv[:, kc, :])
                w_sb[nm] = t
                xm_prefetch(pf + 1)
            if with_qkv_bias:
                qkvb_sb = wpool.tile([1, 3 * FL], XDT, tag="qkvb")
                nc.sync.dma_start(out=qkvb_sb[:], in_=i_qkvb[:])
                ones1 = wpool.tile([1, 128], XDT, tag="ones1")
                nc.sync.dma_start(out=ones1[:], in_=i_ones1[:])
            if with_proj_bias:
                pb_sb = wpool.tile([128, KC], F32, tag="pb")
                nc.sync.dma_start(out=pb_sb[:], in_=i_pb[:])
            ident = wpool.tile([128, 128], BF16, tag="ident")
            make_identity(nc, ident[:])

            qT = pp.tile([128, 4, N], BF16, tag="qT")     # [f%128, fc, n]
            kT = pp.tile([128, 4, N], BF16, tag="kT")
            yT = pp.tile([128, 4, N], BF16, tag="yT")
            vball = pp.tile([128, NCHUNK, HL * 96], BF16, tag="vball")
            # per-head 96-col block: [1 | zeros(31) | v(64)].  Even head h
            # slices [96h+32, 96h+160): oT rows 0-63, sums row 64.  Odd head h
            # slices [96h-32, 96h+96): oT rows 64-127, sums row 32.  Unused
            # psum rows collect garbage and are never read.
            vview = vball[:].rearrange("p jc (h c) -> p jc h c", c=96)
            nc.vector.memset(vview[:, :, :, 0:32], 0.0)
            nc.vector.memset(vview[:, :, :, 0:1], 1.0)

            rope_sb = {}
            for nm2, srct in (("raq", i_raq), ("rbq", i_rbq),
                              ("rak", i_rak), ("rbk", i_rbk)):
                rt = wpool.tile([128, NCHUNK, D], BF16, tag=nm2)
                nc.sync.dma_start(out=rt[:], in_=srct[:].rearrange(
                    "(m p) d -> p m d", p=128))
                rope_sb[nm2] = rt

            wp_sb = wpool.tile([128, 4, C], BF16, tag="wp")
            nc.sync.dma_start(out=wp_sb[:], in_=i_wp[:].rearrange(
                "(kc c) o -> c kc o", c=128))

            # ================= Phase 1: qkv + rmsnorm + rope + transpose ====
            # Software-pipelined across n-chunks: S1 (qkv matmuls + psum
            # copies), S2 (rms stats + rope), S3 (PE transposes) are emitted
            # with a skew so each engine's FIFO interleaves iterations.
            loop_ctx = tc.For_i(0, bench_reps, 1) if bench_reps else None
            if loop_ctx is not None:
                ctx.enter_context(loop_ctx)

            nat_t = {}
            ro_t = {}
            qkv_tags = ("sc0", "sc1", "avA", "avB")

            def stage1(m):
                xm = xm_t.pop(m)
                xm_prefetch(m + 4)
                nat = {"q": natq.tile([128, FL], BF16, tag="qnat", name=f"qnat{m}"),
                       "k": natk.tile([128, FL], BF16, tag="knat", name=f"knat{m}")}
                nat_t[m] = nat
                for ti, nm in enumerate(("q", "k", "v")):
                    psum = psp.tile([128, FL], F32,
                                    tag=qkv_tags[(3 * m + ti) % 4],
                                    name=f"qkvps{m}_{ti}")
                    if with_qkv_bias:
                        nc.tensor.matmul(
                            psum[:], ones1[:],
                            qkvb_sb[:, ti * FL:(ti + 1) * FL],
                            start=True, stop=False)
                    for kc in range(KC):
                        nc.tensor.matmul(psum[:], xm[:, kc, :],
                                         w_sb[nm][:, kc, :],
                                         start=(kc == 0 and not with_qkv_bias),
                                         stop=(kc == KC - 1))
                    if nm == "v":
                        # straight into AV layout [n_p, jc=m, head, d]
                        nc.scalar.copy(
                            out=vview[:, m, :, 32:96],
                            in_=psum[:].rearrange("p (h d) -> p h d", h=HL))
                    else:
                        nc.scalar.copy(out=nat[nm][:], in_=psum[:])

            rstd_t = {}

            def stage2a(m):
                nat = nat_t[m]
                sums = sumsp.tile([128, 2 * HL], F32, tag="sums",
                                  name=f"sums{m}")
                for si, nm in enumerate(("q", "k")):
                    sq = scratch.tile([128, FL], BF16, tag="sxt",
                                      name=f"sq_{m}_{si}")
                    nc.scalar.square(sq[:], nat[nm][:])
                    nc.vector.tensor_reduce(
                        sums[:, si * HL:(si + 1) * HL],
                        sq[:].rearrange("p (h d) -> p h d", h=HL),
                        axis=mybir.AxisListType.X, op=mybir.AluOpType.add)
                rstd_t[m] = sums

            def stage2b(m):
                nsl = slice(m * 128, (m + 1) * 128)
                nat = nat_t.pop(m)
                sums = rstd_t.pop(m)
                # rstd = sqrt(1/(ms + eps)).  Ln+Exp would thrash the ACT
                # table sets against phase 2's Exp; recip on DVE (fast
                # approx, ~18 bits — plenty) + Sqrt keeps one set per phase.
                msum = sumsp.tile([128, 2 * HL], F32, tag="lns", name=f"lns{m}")
                nc.vector.tensor_scalar(
                    msum[:], sums[:], 1.0 / D, EPS,
                    op0=mybir.AluOpType.mult, op1=mybir.AluOpType.add)
                rinv = sumsp.tile([128, 2 * HL], F32, tag="rinv",
                                  name=f"rinv{m}")
                nc.vector.reciprocal_approx_fast(rinv[:], msum[:])
                rstd = sumsp.tile([128, 2 * HL], BF16, tag="rstd",
                                  name=f"rstd{m}")
                nc.scalar.sqrt(rstd[:], rinv[:])
                for si, (nm, ra, rb) in enumerate(
                        (("q", "raq", "rbq"), ("k", "rak", "rbk"))):
                    rat = rope_sb[ra][:, m, :]
                    rbt = rope_sb[rb][:, m, :]

                    xv = nat[nm][:].rearrange("p (h d) -> p h d", h=HL)
                    rsview = bass.AP(
                        tensor=rstd.tensor, offset=rstd[:].offset + si * HL,
                        ap=[rstd[:].ap[0], [1, HL], [0, D]])
                    xn = scratch.tile([128, FL], BF16, tag="sxt",
                                      name=f"xn_{m}_{si}")
                    xnv = xn[:].rearrange("p (h d) -> p h d", h=HL)
                    nc.vector.tensor_mul(xnv, xv, rsview)

                    rav = bass.AP(tensor=rat.tensor, offset=rat.offset,
                                  ap=[rat.ap[0], [0, HL], [1, D]])
                    t1 = scratch.tile([128, FL], BF16, tag="sxt",
                                      name=f"t1_{m}_{si}")
                    t1v = t1[:].rearrange("p (h d) -> p h d", h=HL)
                    nc.gpsimd.tensor_mul(t1v, xnv, rav)

                    ro = scratch.tile([128, FL], BF16, tag="ro", bufs=4,
                                      name=f"ro_{m}_{si}")
                    rov = ro[:].rearrange("p (h d) -> p h d", h=HL)
                    rb_lo = bass.AP(tensor=rbt.tensor, offset=rbt.offset,
                                    ap=[rbt.ap[0], [0, HL], [1, 32]])
                    rb_hi = bass.AP(tensor=rbt.tensor, offset=rbt.offset + 32,
                                    ap=[rbt.ap[0], [0, HL], [1, 32]])
                    nc.vector.tensor_mul(rov[:, :, 0:32], xnv[:, :, 32:64], rb_lo)
                    nc.vector.tensor_mul(rov[:, :, 32:64], xnv[:, :, 0:32], rb_hi)
                    nc.gpsimd.tensor_add(ro[:], ro[:], t1[:])
                    ro_t[(m, si)] = ro

            def stage3(m):
                nsl = slice(m * 128, (m + 1) * 128)
                for si, dst in ((0, qT), (1, kT)):
                    ro = ro_t.pop((m, si))
                    for fc in range(4):
                        tp = psp.tile([128, 128], BF16, tag=f"av{'AB'[fc % 2]}",
                                      name=f"trps{m}_{si}_{fc}")
                        nc.tensor.transpose(tp[:], ro[:, fc * 128:(fc + 1) * 128],
                                            ident[:])
                        if fc % 2:
                            nc.scalar.copy(out=dst[:, fc, nsl], in_=tp[:])
                        else:
                            nc.vector.tensor_copy(out=dst[:, fc, nsl], in_=tp[:])

            for t in range(NCHUNK + 3):
                if t >= 3:
                    stage3(t - 3)
                if 2 <= t < NCHUNK + 2:
                    stage2b(t - 2)
                if 1 <= t < NCHUNK + 1:
                    stage2a(t - 1)
                if t < NCHUNK:
                    stage1(t)

            # ================= Phase 2: attention, flat pipeline ============
            # All 8 (fc, ih) blocks flattened into one 128-step pipeline:
            # step t emits scores+exps of unit t and the AV matmuls of unit
            # t-SKEW (which may belong to the previous block).  Block
            # boundaries disappear from the PE FIFO: the next block's score
            # groups always sit between a block's last AVs, so the PE never
            # drains while ACT catches up, and the psum readout of block b
            # overlaps block b+1's scores.
            PHASES = int(os.environ.get("KERNEL_PHASES", "3"))
            NOSM = bool(os.environ.get("KERNEL_NOSM"))
            P2_SKEW = 2
            blocks = [(fc, ih) for fc in range(4) for ih in range(IH)]
            av_t = {}
            pt_t = {}

            def scores_exp(b, jc):
                fc, ih = blocks[b]
                jsl = slice(jc * 128, (jc + 1) * 128)
                for half in range(2):
                    po = 64 * half
                    sc = psp.tile([128, 1024], F32, tag=f"sc{half}",
                                  name=f"sc{b}_{jc}_{half}")
                    lhs = kT[po:po + 64, fc, jsl]
                    for i2 in range(2):
                        nc.tensor.matmul(
                            sc[:, i2 * 512:(i2 + 1) * 512], lhs,
                            qT[po:po + 64, fc,
                               ih * 1024 + i2 * 512:
                               ih * 1024 + (i2 + 1) * 512],
                            start=True, stop=True)
                    if NOSM:
                        continue
                    pt = ptp.tile([128, 1024], BF16, tag=f"pt{half}",
                                  name=f"pt{b}_{jc}_{half}")
                    nc.scalar.activation(pt[:], sc[:], AF.Exp,
                                         scale=float(SCALE))
                    pt_t[(b, jc, half)] = pt

            def do_av(b, jc):
                fc, ih = blocks[b]
                if jc == 0:
                    av_t[b] = {
                        0: psp.tile([128, 1024], F32, tag="avA",
                                    name=f"avA{b}"),
                        1: psp.tile([128, 1024], F32, tag="avB",
                                    name=f"avB{b}")}
                av = av_t[b]
                for half in range(2):
                    h = 2 * fc + half
                    vs = 96 * h + 32 if half == 0 else 96 * h - 32
                    pt = pt_t.pop((b, jc, half))
                    for i2 in range(2):
                        nc.tensor.matmul(
                            av[half][:, i2 * 512:(i2 + 1) * 512],
                            vball[:, jc, vs:vs + 128],
                            pt[:, i2 * 512:(i2 + 1) * 512],
                            start=(jc == 0), stop=(jc == NCHUNK - 1))

            def normalize(b):
                # yT[f, n] = av_oT[f, n] * (1 / av_sums[n]).  even head: oT
                # rows 0-63, sums row 64; odd head: oT rows 64-127, sums row
                # 32 (lane-aligned by vball layout).  readout on DVE (frees
                # the banks); recip (fast DVE approx, staged to partition 0
                # --- nonzero input base mangles on HW) + gpsimd
                # partition-broadcast; mul back on DVE (gpsimd would thrash
                # Q7 libraries against partition_broadcast).
                fc, ih = blocks[b]
                isl = slice(ih * 1024, (ih + 1) * 1024)
                av = av_t.pop(b)
                if os.environ.get("KERNEL_NONORM"):
                    return
                last = b == len(blocks) - 1
                for half in range(2):
                    po = 64 * half
                    srow = 64 if half == 0 else 32
                    avs = rlp.tile([128, 1024], F32, tag="avs", bufs=avb,
                                   name=f"avs{b}_{half}")
                    r0 = rlp.tile([1, 1024], F32, tag="r0",
                                  name=f"r0{b}_{half}")
                    # for the final block ACT is idle (exps done): split the
                    # readout across ACT+DVE to shorten the phase-3 entry gap
                    cp0 = nc.scalar.copy if last else nc.vector.tensor_copy
                    if half == 0:
                        cp0(out=avs[0:65, :], in_=av[half][0:65, :])
                        nc.vector.tensor_copy(out=r0[:], in_=avs[64:65, :])
                    else:
                        cp0(out=avs[64:128, :], in_=av[half][64:128, :])
                        nc.vector.tensor_copy(out=r0[:],
                                              in_=av[half][srow:srow + 1, :])
                    r_l = rlp.tile([1, 1024], F32, tag="rl",
                                   name=f"rl{b}_{half}")
                    nc.vector.reciprocal_approx_fast(r_l[:], r0[:])
                    rbc = rbp.tile([128, 1024], F32, tag="rbc",
                                   name=f"rbc{b}_{half}")
                    nc.gpsimd.partition_broadcast(rbc[:], r_l[:],
                                                  channels=128)
                    nc.vector.tensor_mul(yT[po:po + 64, fc, isl],
                                         avs[po:po + 64, :],
                                         rbc[po:po + 64, :])

            NBLK = len(blocks) if PHASES >= 2 else 0
            for t in range(NBLK * NCHUNK + (P2_SKEW if NBLK else 0)):
                if t < NBLK * NCHUNK:
                    b, jc = divmod(t, NCHUNK)
                    scores_exp(b, jc)
                if t >= P2_SKEW and not NOSM:
                    b2, jc2 = divmod(t - P2_SKEW, NCHUNK)
                    do_av(b2, jc2)
                    if jc2 == NCHUNK - 1:
                        normalize(b2)

            # ================= Phase 3: output projection ===================
            if PHASES < 3:
                nc.sync.dma_start(out=o_FT[:].rearrange('c n -> (c n)')[0:C * N // (2 if qkv_bf16 else 1)].bitcast(XDT), in_=i_xT[:].rearrange('m c kc n -> (m c kc n)'))
            for cc in range(KC if PHASES >= 3 else 0):
                for nt in range(4):
                    fp = psp.tile([128, 512], F32,
                                  tag=("sc0", "sc1", "avA", "avB")[(cc * 4 + nt) % 4],
                                  name=f"fp{cc}_{nt}")
                    for kc in range(4):
                        nc.tensor.matmul(
                            fp[:], wp_sb[:, kc, cc * 128:(cc + 1) * 128],
                            yT[:, kc, nt * 512:(nt + 1) * 512],
                            start=(kc == 0), stop=(kc == 3))
                    so = outp.tile([128, 512], F32, tag="so")
                    if with_proj_bias:
                        nc.scalar.activation(so[:], fp[:], AF.Identity,
                                             bias=pb_sb[:, cc:cc + 1])
                    elif (cc * 4 + nt) % 2:
                        nc.scalar.copy(out=so[:], in_=fp[:])
                    else:
                        nc.vector.tensor_copy(out=so[:], in_=fp[:])
                    nc.sync.dma_start(
                        out=o_FT[cc * 128:(cc + 1) * 128,
                                 nt * 512:(nt + 1) * 512],
                        in_=so[:])

    nc.compile()
    return nc


def _host_prep(x, qkv_w, qkv_b, proj_w, proj_b, qn_w, kn_w, rope_cos, rope_sin,
               qkv_bf16=True):
    xdt = ml_dtypes.bfloat16 if qkv_bf16 else np.float32
    """Build the 8 per-core input maps."""
    x = np.asarray(x, dtype=np.float32)
    qkv_w = np.asarray(qkv_w, dtype=np.float32)
    qkv_b = np.asarray(qkv_b, dtype=np.float32)
    proj_w = np.asarray(proj_w, dtype=np.float32)
    proj_b = np.asarray(proj_b, dtype=np.float32)
    qn_w = np.asarray(qn_w, dtype=np.float32)
    kn_w = np.asarray(kn_w, dtype=np.float32)
    rope_cos = np.asarray(rope_cos, dtype=np.float32)
    rope_sin = np.asarray(rope_sin, dtype=np.float32)

    # rope tables with rmsnorm weight folded in:
    # out[0:32]  = xh[0:32]*(w*cos)[0:32]  + xh[32:64]*(-w2*sin[0:32])
    # out[32:64] = xh[32:64]*(w*cos)[32:64] + xh[0:32]*( w1*sin[32:64])
    def tables(w):
        a = rope_cos * w[None, :]
        b = np.empty_like(rope_sin)
        b[:, 0:32] = -rope_sin[:, 0:32] * w[None, 32:64]
        b[:, 32:64] = rope_sin[:, 32:64] * w[None, 0:32]
        return (np.ascontiguousarray(a).astype(ml_dtypes.bfloat16),
                np.ascontiguousarray(b).astype(ml_dtypes.bfloat16))

    raq, rbq = tables(qn_w)
    rak, rbk = tables(kn_w)
    with_qkv_bias = bool(np.any(qkv_b))
    with_proj_bias = bool(np.any(proj_b))
    ones1 = np.ones((1, 128), dtype=np.float32)
    pb = np.ascontiguousarray(proj_b.reshape(KC, 128).T)

    # shared tensors are computed once per batch / head-half, not per core
    xT = {}
    for b in range(4):
        xT[b] = np.ascontiguousarray(
            x[b].T.reshape(KC, 128, NCHUNK, 128).transpose(2, 1, 0, 3)
        ).astype(xdt)
    wmaps = {}
    for hh in range(2):
        fsl = slice(hh * FL, hh * FL + FL)
        wmaps[hh] = {
            "wq": np.ascontiguousarray(qkv_w[fsl, :].T).astype(xdt),
            "wk": np.ascontiguousarray(qkv_w[C:][fsl, :].T).astype(xdt),
            "wv": np.ascontiguousarray(qkv_w[2 * C:][fsl, :].T).astype(xdt),
            "wp": np.ascontiguousarray(proj_w[:, fsl].T).astype(ml_dtypes.bfloat16),
        }
        if with_qkv_bias:
            wmaps[hh]["qkvb"] = np.concatenate(
                [qkv_b[fsl], qkv_b[C:][fsl], qkv_b[2 * C:][fsl]]
            ).reshape(1, 3 * FL).astype(xdt)

    in_maps = []
    for ci in range(8):
        b, hh = divmod(ci, 2)
        m = {
            "xT": xT[b],
            "raq": raq, "rbq": rbq, "rak": rak, "rbk": rbk,
            **wmaps[hh],
        }
        if with_qkv_bias:
            m["ones1"] = ones1.astype(xdt)
        if with_proj_bias:
            m["pb"] = pb
        in_maps.append(m)
    return in_maps, with_qkv_bias, with_proj_bias


def kernel(x, qkv_w, qkv_b, proj_w, proj_b, qn_w, kn_w, rope_cos, rope_sin,
           _trace=False):
    global _PROGRAM
    in_maps, wqb, wpb = _host_prep(x, qkv_w, qkv_b, proj_w, proj_b, qn_w, kn_w,
                                   rope_cos, rope_sin)
    if _PROGRAM is None or _PROGRAM[0] != (wqb, wpb):
        _PROGRAM = ((wqb, wpb), _build_program(wqb, wpb))
    nc = _PROGRAM[1]
    kwargs = {}
    if _trace:
        kwargs = dict(trace=True, trace_cores=[0])
    res = run_bass_kernel_spmd(nc, in_maps, core_ids=list(range(8)), **kwargs)
    if _trace:
        kernel.last_exec_ns = res.exec_time_ns
        kernel.last_results = res
    out = np.empty((B, N, C), dtype=np.float32)
    for b in range(B):
        ft = res.results[2 * b]["FT"] + res.results[2 * b + 1]["FT"]
        out[b] = ft.T
    return out



# revision 35
# speedup vs baseline: 1.2160x; 1.2160x over previous
"""Fused multi-head attention block (qkv + rmsnorm + rope + sdpa + proj) for
Trainium2, sharded over 8 NeuronCores as batch x head-half (Megatron-style).

Shapes (hardcoded): B=4, N=2048, C=1024, H=16, D=64.
Each core handles one batch and 8 heads (= 512 feature columns).
Host folds qn_w/kn_w into rope tables, sums the two per-batch partial
projection outputs and transposes back to [B, N, C].
"""
import os
import sys

os.environ.setdefault("NEURON_RT_RESET_CORES", "1")
sys.path.insert(0, "/opt/trn_rl_repo")

import ml_dtypes
import numpy as np

import concourse.bass as bass
import concourse.mybir as mybir
import concourse.tile as tile
from concourse import bacc
from concourse.bass_utils import run_bass_kernel_spmd
from concourse.masks import make_identity

dt = mybir.dt
F32 = dt.float32
F32R = dt.float32r
BF16 = dt.bfloat16
AF = mybir.ActivationFunctionType

B, N, C, H, D = 4, 2048, 1024, 16, 64
HL = H // 2          # heads per core = 8
FL = HL * D          # local features = 512
EPS = 1e-6
SCALE = 1.0 / np.sqrt(D)
NCHUNK = N // 128    # 16
KC = C // 128        # 8  (c_in chunks)
IH = 2               # i-halves of 1024 in phase 2
# Schraudolph offload: exp units (2*jc+half) % SCHRAUD == 1 go to DVE.
# 0 disables; 4 = 25% of units, 3 = 33%, 2 = 50%.
SCHRAUD = int(os.environ.get("KERNEL_SCHRAUD", "0"))
SCHRAUD_C = float(os.environ.get("KERNEL_SCHRAUD_C", "0.0434"))
S_A = 128.0 * 1.4426950408889634 * (1.0 / np.sqrt(64))
S_B = 128.0 * (127.0 - SCHRAUD_C)

_PROGRAM = None


def _build_program(with_qkv_bias, with_proj_bias, bench_reps=0, qkv_bf16=True):
    XDT = BF16 if qkv_bf16 else F32R
    xlb, nqb, scb, ptb, avb = (5, 6, 10, 4, 2) if qkv_bf16 else (3, 4, 8, 3, 1)
    nc = bacc.Bacc("TRN2", target_bir_lowering=False, debug=False, num_devices=8)

    i_xT = nc.dram_tensor("xT", [NCHUNK, 128, KC, 128], XDT, kind="ExternalInput")
    i_wq = nc.dram_tensor("wq", [C, FL], XDT, kind="ExternalInput")
    i_wk = nc.dram_tensor("wk", [C, FL], XDT, kind="ExternalInput")
    i_wv = nc.dram_tensor("wv", [C, FL], XDT, kind="ExternalInput")
    i_wp = nc.dram_tensor("wp", [FL, C], BF16, kind="ExternalInput")
    if with_qkv_bias:
        i_qkvb = nc.dram_tensor("qkvb", [1, 3 * FL], XDT, kind="ExternalInput")
        i_ones1 = nc.dram_tensor("ones1", [1, 128], XDT, kind="ExternalInput")
    i_raq = nc.dram_tensor("raq", [N, D], BF16, kind="ExternalInput")
    i_rbq = nc.dram_tensor("rbq", [N, D], BF16, kind="ExternalInput")
    i_rak = nc.dram_tensor("rak", [N, D], BF16, kind="ExternalInput")
    i_rbk = nc.dram_tensor("rbk", [N, D], BF16, kind="ExternalInput")
    if with_proj_bias:
        i_pb = nc.dram_tensor("pb", [128, KC], F32, kind="ExternalInput")
    o_FT = nc.dram_tensor("FT", [C, N], F32, kind="ExternalOutput")

    from contextlib import ExitStack
    with tile.TileContext(nc) as tc:
        with ExitStack() as ctx:
            pp = ctx.enter_context(tc.tile_pool(name="persist", bufs=1))
            wpool = ctx.enter_context(tc.tile_pool(name="wpool", bufs=1))
            xload = ctx.enter_context(tc.tile_pool(name="xload", bufs=xlb))
            scratch = ctx.enter_context(tc.tile_pool(name="scratch", bufs=scb))
            natq = ctx.enter_context(tc.tile_pool(name="natq", bufs=nqb))
            natk = ctx.enter_context(tc.tile_pool(name="natk", bufs=nqb))
            sumsp = ctx.enter_context(tc.tile_pool(name="sumsp", bufs=4))
            ptp = ctx.enter_context(tc.tile_pool(name="ptp", bufs=ptb))
            rlp = ctx.enter_context(tc.tile_pool(name="rlp", bufs=2))
            rbp = ctx.enter_context(tc.tile_pool(name="rbp", bufs=2))
            outp = ctx.enter_context(tc.tile_pool(name="outp", bufs=4))
            # one PSUM pool, 4 tags x 2 banks (8 banks total), shared by all
            # phases: scA/scB + avA/avB
            psp = ctx.enter_context(tc.tile_pool(name="psp", bufs=1, space="PSUM"))

            # ---- persistent tensors ----
            # DMA emission order matters: the sync queue drains in order, so
            # interleave the first x chunks with the qkv weights (q first)
            # and push wp (phase-3-only) to the back.
            xm_t = {}

            def xm_prefetch(m):
                if m >= NCHUNK:
                    return
                xm = xload.tile([128, KC, 128], XDT, tag="xm", name=f"xm{m}")
                nc.sync.dma_start(out=xm[:], in_=i_xT[m])
                xm_t[m] = xm

            xm_prefetch(0)
            w_sb = {}
            # per-kc weight chunks: the m=0 qkv matmuls accumulate kc=0..7
            # in order, so they can chase the chunk DMAs instead of waiting
            # for the full 1MB tile
            for pf, (nm, src) in enumerate(
                    (("q", i_wq), ("k", i_wk), ("v", i_wv))):
                t = wpool.tile([128, KC, FL], XDT, tag=f"w{nm}")
                srcv = src[:].rearrange("(kc c) f -> c kc f", c=128)
                for kc in range(KC):
                    nc.sync.dma_start(out=t[:, kc, :], in_=srcv[:, kc, :])
                w_sb[nm] = t
                xm_prefetch(pf + 1)
            if with_qkv_bias:
                qkvb_sb = wpool.tile([1, 3 * FL], XDT, tag="qkvb")
                nc.sync.dma_start(out=qkvb_sb[:], in_=i_qkvb[:])
                ones1 = wpool.tile([1, 128], XDT, tag="ones1")
                nc.sync.dma_start(out=ones1[:], in_=i_ones1[:])
            if with_proj_bias:
                pb_sb = wpool.tile([128, KC], F32, tag="pb")
                nc.sync.dma_start(out=pb_sb[:], in_=i_pb[:])
            ident = wpool.tile([128, 128], BF16, tag="ident")
            make_identity(nc, ident[:])

            qT = pp.tile([128, 4, N], BF16, tag="qT")     # [f%128, fc, n]
            kT = pp.tile([128, 4, N], BF16, tag="kT")
            yT = pp.tile([128, 4, N], BF16, tag="yT")
            vball = pp.tile([128, NCHUNK, HL * 96], BF16, tag="vball")
            # per-head 96-col block: [1 | zeros(31) | v(64)].  Even head h
            # slices [96h+32, 96h+160): oT rows 0-63, sums row 64.  Odd head h
            # slices [96h-32, 96h+96): oT rows 64-127, sums row 32.  Unused
            # psum rows collect garbage and are never read.
            vview = vball[:].rearrange("p jc (h c) -> p jc h c", c=96)
            nc.vector.memset(vview[:, :, :, 0:32], 0.0)
            nc.vector.memset(vview[:, :, :, 0:1], 1.0)

            rope_sb = {}
            for nm2, srct in (("raq", i_raq), ("rbq", i_rbq),
                              ("rak", i_rak), ("rbk", i_rbk)):
                rt = wpool.tile([128, NCHUNK, D], BF16, tag=nm2)
                nc.sync.dma_start(out=rt[:], in_=srct[:].rearrange(
                    "(m p) d -> p m d", p=128))
                rope_sb[nm2] = rt

            wp_sb = wpool.tile([128, 4, C], BF16, tag="wp")
            nc.sync.dma_start(out=wp_sb[:], in_=i_wp[:].rearrange(
                "(kc c) o -> c kc o", c=128))

            # ================= Phase 1: qkv + rmsnorm + rope + transpose ====
            # Software-pipelined across n-chunks: S1 (qkv matmuls + psum
            # copies), S2 (rms stats + rope), S3 (PE transposes) are emitted
            # with a skew so each engine's FIFO interleaves iterations.
            loop_ctx = tc.For_i(0, bench_reps, 1) if bench_reps else None
            if loop_ctx is not None:
                ctx.enter_context(loop_ctx)

            nat_t = {}
            ro_t = {}
            qkv_tags = ("sc0", "sc1", "avA", "avB")

            def stage1(m):
                xm = xm_t.pop(m)
                xm_prefetch(m + 4)
                nat = {"q": natq.tile([128, FL], BF16, tag="qnat", name=f"qnat{m}"),
                       "k": natk.tile([128, FL], BF16, tag="knat", name=f"knat{m}")}
                nat_t[m] = nat
                for ti, nm in enumerate(("q", "k", "v")):
                    psum = psp.tile([128, FL], F32,
                                    tag=qkv_tags[(3 * m + ti) % 4],
                                    name=f"qkvps{m}_{ti}")
                    if with_qkv_bias:
                        nc.tensor.matmul(
                            psum[:], ones1[:],
                            qkvb_sb[:, ti * FL:(ti + 1) * FL],
                            start=True, stop=False)
                    for kc in range(KC):
                        nc.tensor.matmul(psum[:], xm[:, kc, :],
                                         w_sb[nm][:, kc, :],
                                         start=(kc == 0 and not with_qkv_bias),
                                         stop=(kc == KC - 1))
                    if nm == "v":
                        # straight into AV layout [n_p, jc=m, head, d]
                        nc.scalar.copy(
                            out=vview[:, m, :, 32:96],
                            in_=psum[:].rearrange("p (h d) -> p h d", h=HL))
                    else:
                        nc.scalar.copy(out=nat[nm][:], in_=psum[:])

            rstd_t = {}

            def stage2a(m):
                nat = nat_t[m]
                sums = sumsp.tile([128, 2 * HL], F32, tag="sums",
                                  name=f"sums{m}")
                for si, nm in enumerate(("q", "k")):
                    sq = scratch.tile([128, FL], BF16, tag="sxt",
                                      name=f"sq_{m}_{si}")
                    nc.gpsimd.tensor_mul(sq[:], nat[nm][:], nat[nm][:])
                    nc.vector.tensor_reduce(
                        sums[:, si * HL:(si + 1) * HL],
                        sq[:].rearrange("p (h d) -> p h d", h=HL),
                        axis=mybir.AxisListType.X, op=mybir.AluOpType.add)
                rstd_t[m] = sums

            def stage2b(m):
                nsl = slice(m * 128, (m + 1) * 128)
                nat = nat_t.pop(m)
                sums = rstd_t.pop(m)
                # rstd = sqrt(1/(ms + eps)).  Ln+Exp would thrash the ACT
                # table sets against phase 2's Exp; recip on DVE (fast
                # approx, ~18 bits — plenty) + Sqrt keeps one set per phase.
                msum = sumsp.tile([128, 2 * HL], F32, tag="lns", name=f"lns{m}")
                nc.vector.tensor_scalar(
                    msum[:], sums[:], 1.0 / D, EPS,
                    op0=mybir.AluOpType.mult, op1=mybir.AluOpType.add)
                rinv = sumsp.tile([128, 2 * HL], F32, tag="rinv",
                                  name=f"rinv{m}")
                nc.vector.reciprocal_approx_fast(rinv[:], msum[:])
                rstd = sumsp.tile([128, 2 * HL], BF16, tag="rstd",
                                  name=f"rstd{m}")
                nc.scalar.sqrt(rstd[:], rinv[:])
                for si, (nm, ra, rb) in enumerate(
                        (("q", "raq", "rbq"), ("k", "rak", "rbk"))):
                    rat = rope_sb[ra][:, m, :]
                    rbt = rope_sb[rb][:, m, :]

                    xv = nat[nm][:].rearrange("p (h d) -> p h d", h=HL)
                    rsview = bass.AP(
                        tensor=rstd.tensor, offset=rstd[:].offset + si * HL,
                        ap=[rstd[:].ap[0], [1, HL], [0, D]])
                    xn = scratch.tile([128, FL], BF16, tag="sxt",
                                      name=f"xn_{m}_{si}")
                    xnv = xn[:].rearrange("p (h d) -> p h d", h=HL)
                    nc.vector.tensor_mul(xnv, xv, rsview)

                    rav = bass.AP(tensor=rat.tensor, offset=rat.offset,
                                  ap=[rat.ap[0], [0, HL], [1, D]])
                    t1 = scratch.tile([128, FL], BF16, tag="sxt",
                                      name=f"t1_{m}_{si}")
                    t1v = t1[:].rearrange("p (h d) -> p h d", h=HL)
                    nc.vector.tensor_mul(t1v, xnv, rav)

                    ro = scratch.tile([128, FL], BF16, tag="ro", bufs=4,
                                      name=f"ro_{m}_{si}")
                    rov = ro[:].rearrange("p (h d) -> p h d", h=HL)
                    rb_lo = bass.AP(tensor=rbt.tensor, offset=rbt.offset,
                                    ap=[rbt.ap[0], [0, HL], [1, 32]])
                    rb_hi = bass.AP(tensor=rbt.tensor, offset=rbt.offset + 32,
                                    ap=[rbt.ap[0], [0, HL], [1, 32]])
                    nc.vector.tensor_mul(rov[:, :, 0:32], xnv[:, :, 32:64], rb_lo)
                    nc.vector.tensor_mul(rov[:, :, 32:64], xnv[:, :, 0:32], rb_hi)
                    nc.gpsimd.tensor_add(ro[:], ro[:], t1[:])
                    ro_t[(m, si)] = ro

            def stage3(m):
                nsl = slice(m * 128, (m + 1) * 128)
                for si, dst in ((0, qT), (1, kT)):
                    ro = ro_t.pop((m, si))
                    for fc in range(4):
                        tp = psp.tile([128, 128], BF16, tag=f"av{'AB'[fc % 2]}",
                                      name=f"trps{m}_{si}_{fc}")
                        nc.tensor.transpose(tp[:], ro[:, fc * 128:(fc + 1) * 128],
                                            ident[:])
                        if fc % 2:
                            nc.scalar.copy(out=dst[:, fc, nsl], in_=tp[:])
                        else:
                            nc.vector.tensor_copy(out=dst[:, fc, nsl], in_=tp[:])

            for t in range(NCHUNK + 3):
                if t >= 3:
                    stage3(t - 3)
                if 2 <= t < NCHUNK + 2:
                    stage2b(t - 2)
                if 1 <= t < NCHUNK + 1:
                    stage2a(t - 1)
                if t < NCHUNK:
                    stage1(t)

            # ================= Phase 2: attention, flat pipeline ============
            # All 8 (fc, ih) blocks flattened into one 128-step pipeline:
            # step t emits scores+exps of unit t and the AV matmuls of unit
            # t-SKEW (which may belong to the previous block).  Block
            # boundaries disappear from the PE FIFO: the next block's score
            # groups always sit between a block's last AVs, so the PE never
            # drains while ACT catches up, and the psum readout of block b
            # overlaps block b+1's scores.
            PHASES = int(os.environ.get("KERNEL_PHASES", "3"))
            NOSM = bool(os.environ.get("KERNEL_NOSM"))
            P2_SKEW = 2
            blocks = [(fc, ih) for fc in range(4) for ih in range(IH)]
            av_t = {}
            pt_t = {}

            def scores_exp(b, jc):
                fc, ih = blocks[b]
                jsl = slice(jc * 128, (jc + 1) * 128)
                for half in range(2):
                    po = 64 * half
                    sc = psp.tile([128, 1024], F32, tag=f"sc{half}",
                                  name=f"sc{b}_{jc}_{half}")
                    lhs = kT[po:po + 64, fc, jsl]
                    for i2 in range(2):
                        nc.tensor.matmul(
                            sc[:, i2 * 512:(i2 + 1) * 512], lhs,
                            qT[po:po + 64, fc,
                               ih * 1024 + i2 * 512:
                               ih * 1024 + (i2 + 1) * 512],
                            start=True, stop=True)
                    if NOSM:
                        continue
                    pt = ptp.tile([128, 1024], BF16, tag=f"pt{half}",
                                  name=f"pt{b}_{jc}_{half}")
                    nc.scalar.activation(pt[:], sc[:], AF.Exp,
                                         scale=float(SCALE))
                    pt_t[(b, jc, half)] = pt

            def do_av(b, jc):
                fc, ih = blocks[b]
                if jc == 0:
                    av_t[b] = {
                        0: psp.tile([128, 1024], F32, tag="avA",
                                    name=f"avA{b}"),
                        1: psp.tile([128, 1024], F32, tag="avB",
                                    name=f"avB{b}")}
                av = av_t[b]
                for half in range(2):
                    h = 2 * fc + half
                    vs = 96 * h + 32 if half == 0 else 96 * h - 32
                    pt = pt_t.pop((b, jc, half))
                    for i2 in range(2):
                        nc.tensor.matmul(
                            av[half][:, i2 * 512:(i2 + 1) * 512],
                            vball[:, jc, vs:vs + 128],
                            pt[:, i2 * 512:(i2 + 1) * 512],
                            start=(jc == 0), stop=(jc == NCHUNK - 1))

            def normalize(b):
                # yT[f, n] = av_oT[f, n] * (1 / av_sums[n]).  even head: oT
                # rows 0-63, sums row 64; odd head: oT rows 64-127, sums row
                # 32 (lane-aligned by vball layout).  readout on DVE (frees
                # the banks); recip (fast DVE approx, staged to partition 0
                # --- nonzero input base mangles on HW) + gpsimd
                # partition-broadcast; mul back on DVE (gpsimd would thrash
                # Q7 libraries against partition_broadcast).
                fc, ih = blocks[b]
                isl = slice(ih * 1024, (ih + 1) * 1024)
                av = av_t.pop(b)
                if os.environ.get("KERNEL_NONORM"):
                    return
                for half in range(2):
                    po = 64 * half
                    srow = 64 if half == 0 else 32
                    avs = rlp.tile([128, 1024], F32, tag="avs", bufs=avb,
                                   name=f"avs{b}_{half}")
                    r0 = rlp.tile([1, 1024], F32, tag="r0",
                                  name=f"r0{b}_{half}")
                    if half == 0:
                        nc.vector.tensor_copy(out=avs[0:65, :],
                                              in_=av[half][0:65, :])
                        nc.vector.tensor_copy(out=r0[:], in_=avs[64:65, :])
                    else:
                        nc.vector.tensor_copy(out=avs[64:128, :],
                                              in_=av[half][64:128, :])
                        nc.vector.tensor_copy(out=r0[:],
                                              in_=av[half][srow:srow + 1, :])
                    r_l = rlp.tile([1, 1024], F32, tag="rl",
                                   name=f"rl{b}_{half}")
                    nc.vector.reciprocal_approx_fast(r_l[:], r0[:])
                    rbc = rbp.tile([128, 1024], F32, tag="rbc",
                                   name=f"rbc{b}_{half}")
                    nc.gpsimd.partition_broadcast(rbc[:], r_l[:],
                                                  channels=128)
                    nc.vector.tensor_mul(yT[po:po + 64, fc, isl],
                                         avs[po:po + 64, :],
                                         rbc[po:po + 64, :])

            NBLK = len(blocks) if PHASES >= 2 else 0
            for t in range(NBLK * NCHUNK + (P2_SKEW if NBLK else 0)):
                if t < NBLK * NCHUNK:
                    b, jc = divmod(t, NCHUNK)
                    scores_exp(b, jc)
                if t >= P2_SKEW and not NOSM:
                    b2, jc2 = divmod(t - P2_SKEW, NCHUNK)
                    do_av(b2, jc2)
                    if jc2 == NCHUNK - 1:
                        normalize(b2)

            # ================= Phase 3: output projection ===================
            if PHASES < 3:
                nc.sync.dma_start(out=o_FT[:].rearrange('c n -> (c n)')[0:C * N // (2 if qkv_bf16 else 1)].bitcast(XDT), in_=i_xT[:].rearrange('m c kc n -> (m c kc n)'))
            for cc in range(KC if PHASES >= 3 else 0):
                for nt in range(4):
                    fp = psp.tile([128, 512], F32,
                                  tag=("sc0", "sc1", "avA", "avB")[(cc * 4 + nt) % 4],
                                  name=f"fp{cc}_{nt}")
                    for kc in range(4):
                        nc.tensor.matmul(
                            fp[:], wp_sb[:, kc, cc * 128:(cc + 1) * 128],
                            yT[:, kc, nt * 512:(nt + 1) * 512],
                            start=(kc == 0), stop=(kc == 3))
                    so = outp.tile([128, 512], F32, tag="so")
                    if with_proj_bias:
                        nc.scalar.activation(so[:], fp[:], AF.Identity,
                                             bias=pb_sb[:, cc:cc + 1])
                    elif (cc * 4 + nt) % 2:
                        nc.scalar.copy(out=so[:], in_=fp[:])
                    else:
                        nc.vector.tensor_copy(out=so[:], in_=fp[:])
                    nc.sync.dma_start(
                        out=o_FT[cc * 128:(cc + 1) * 128,
                                 nt * 512:(nt + 1) * 512],
                        in_=so[:])

    nc.compile()
    return nc


def _host_prep(x, qkv_w, qkv_b, proj_w, proj_b, qn_w, kn_w, rope_cos, rope_sin,
               qkv_bf16=True):
    xdt = ml_dtypes.bfloat16 if qkv_bf16 else np.float32
    """Build the 8 per-core input maps."""
    x = np.asarray(x, dtype=np.float32)
    qkv_w = np.asarray(qkv_w, dtype=np.float32)
    qkv_b = np.asarray(qkv_b, dtype=np.float32)
    proj_w = np.asarray(proj_w, dtype=np.float32)
    proj_b = np.asarray(proj_b, dtype=np.float32)
    qn_w = np.asarray(qn_w, dtype=np.float32)
    kn_w = np.asarray(kn_w, dtype=np.float32)
    rope_cos = np.asarray(rope_cos, dtype=np.float32)
    rope_sin = np.asarray(rope_sin, dtype=np.float32)

    # rope tables with rmsnorm weight folded in:
    # out[0:32]  = xh[0:32]*(w*cos)[0:32]  + xh[32:64]*(-w2*sin[0:32])
    # out[32:64] = xh[32:64]*(w*cos)[32:64] + xh[0:32]*( w1*sin[32:64])
    def tables(w):
        a = rope_cos * w[None, :]
        b = np.empty_like(rope_sin)
        b[:, 0:32] = -rope_sin[:, 0:32] * w[None, 32:64]
        b[:, 32:64] = rope_sin[:, 32:64] * w[None, 0:32]
        return (np.ascontiguousarray(a).astype(ml_dtypes.bfloat16),
                np.ascontiguousarray(b).astype(ml_dtypes.bfloat16))

    raq, rbq = tables(qn_w)
    rak, rbk = tables(kn_w)
    with_qkv_bias = bool(np.any(qkv_b))
    with_proj_bias = bool(np.any(proj_b))
    ones1 = np.ones((1, 128), dtype=np.float32)
    pb = np.ascontiguousarray(proj_b.reshape(KC, 128).T)

    # shared tensors are computed once per batch / head-half, not per core
    xT = {}
    for b in range(4):
        xT[b] = np.ascontiguousarray(
            x[b].T.reshape(KC, 128, NCHUNK, 128).transpose(2, 1, 0, 3)
        ).astype(xdt)
    wmaps = {}
    for hh in range(2):
        fsl = slice(hh * FL, hh * FL + FL)
        wmaps[hh] = {
            "wq": np.ascontiguousarray(qkv_w[fsl, :].T).astype(xdt),
            "wk": np.ascontiguousarray(qkv_w[C:][fsl, :].T).astype(xdt),
            "wv": np.ascontiguousarray(qkv_w[2 * C:][fsl, :].T).astype(xdt),
            "wp": np.ascontiguousarray(proj_w[:, fsl].T).astype(ml_dtypes.bfloat16),
        }
        if with_qkv_bias:
            wmaps[hh]["qkvb"] = np.concatenate(
                [qkv_b[fsl], qkv_b[C:][fsl], qkv_b[2 * C:][fsl]]
            ).reshape(1, 3 * FL).astype(xdt)

    in_maps = []
    for ci in range(8):
        b, hh = divmod(ci, 2)
        m = {
            "xT": xT[b],
            "raq": raq, "rbq": rbq, "rak": rak, "rbk": rbk,
            **wmaps[hh],
        }
        if with_qkv_bias:
            m["ones1"] = ones1.astype(xdt)
        if with_proj_bias:
            m["pb"] = pb
        in_maps.append(m)
    return in_maps, with_qkv_bias, with_proj_bias


def kernel(x, qkv_w, qkv_b, proj_w, proj_b, qn_w, kn_w, rope_cos, rope_sin,
           _trace=False):
    global _PROGRAM
    in_maps, wqb, wpb = _host_prep(x, qkv_w, qkv_b, proj_w, proj_b, qn_w, kn_w,
                                   rope_cos, rope_sin)
    if _PROGRAM is None or _PROGRAM[0] != (wqb, wpb):
        _PROGRAM = ((wqb, wpb), _build_program(wqb, wpb))
    nc = _PROGRAM[1]
    kwargs = {}
    if _trace:
        kwargs = dict(trace=True, trace_cores=[0])
    res = run_bass_kernel_spmd(nc, in_maps, core_ids=list(range(8)), **kwargs)
    if _trace:
        kernel.last_exec_ns = res.exec_time_ns
        kernel.last_results = res
    out = np.empty((B, N, C), dtype=np.float32)
    for b in range(B):
        ft = res.results[2 * b]["FT"] + res.results[2 * b + 1]["FT"]
        out[b] = ft.T
    return out



# revision 37
# speedup vs baseline: 1.2250x; 1.0074x over previous
"""Fused multi-head attention block (qkv + rmsnorm + rope + sdpa + proj) for
Trainium2, sharded over 8 NeuronCores as batch x head-half (Megatron-style).

Shapes (hardcoded): B=4, N=2048, C=1024, H=16, D=64.
Each core handles one batch and 8 heads (= 512 feature columns).
Host folds qn_w/kn_w into rope tables, sums the two per-batch partial
projection outputs and transposes back to [B, N, C].
"""
import os
import sys

os.environ.setdefault("NEURON_RT_RESET_CORES", "1")
sys.path.insert(0, "/opt/trn_rl_repo")

import ml_dtypes
import numpy as np

import concourse.bass as bass
import concourse.mybir as mybir
import concourse.tile as tile
from concourse import bacc
from concourse.bass_utils import run_bass_kernel_spmd
from concourse.masks import make_identity

dt = mybir.dt
F32 = dt.float32
F32R = dt.float32r
BF16 = dt.bfloat16
AF = mybir.ActivationFunctionType

B, N, C, H, D = 4, 2048, 1024, 16, 64
HL = H // 2          # heads per core = 8
FL = HL * D          # local features = 512
EPS = 1e-6
SCALE = 1.0 / np.sqrt(D)
NCHUNK = N // 128    # 16
KC = C // 128        # 8  (c_in chunks)
IH = 2               # i-halves of 1024 in phase 2
# Schraudolph offload: exp units (2*jc+half) % SCHRAUD == 1 go to DVE.
# 0 disables; 4 = 25% of units, 3 = 33%, 2 = 50%.
SCHRAUD = int(os.environ.get("KERNEL_SCHRAUD", "0"))
SCHRAUD_C = float(os.environ.get("KERNEL_SCHRAUD_C", "0.0434"))
S_A = 128.0 * 1.4426950408889634 * (1.0 / np.sqrt(64))
S_B = 128.0 * (127.0 - SCHRAUD_C)

_PROGRAM = None


def _build_program(with_qkv_bias, with_proj_bias, bench_reps=0, qkv_bf16=True):
    XDT = BF16 if qkv_bf16 else F32R
    xlb, nqb, scb, ptb, avb = (5, 6, 10, 4, 2) if qkv_bf16 else (3, 4, 8, 3, 1)
    nc = bacc.Bacc("TRN2", target_bir_lowering=False, debug=False, num_devices=8)

    i_xT = nc.dram_tensor("xT", [NCHUNK, 128, KC, 128], XDT, kind="ExternalInput")
    i_wq = nc.dram_tensor("wq", [C, FL], XDT, kind="ExternalInput")
    i_wk = nc.dram_tensor("wk", [C, FL], XDT, kind="ExternalInput")
    i_wv = nc.dram_tensor("wv", [C, FL], XDT, kind="ExternalInput")
    i_wp = nc.dram_tensor("wp", [FL, C], BF16, kind="ExternalInput")
    if with_qkv_bias:
        i_qkvb = nc.dram_tensor("qkvb", [1, 3 * FL], XDT, kind="ExternalInput")
        i_ones1 = nc.dram_tensor("ones1", [1, 128], XDT, kind="ExternalInput")
    i_raq = nc.dram_tensor("raq", [N, D], BF16, kind="ExternalInput")
    i_rbq = nc.dram_tensor("rbq", [N, D], BF16, kind="ExternalInput")
    i_rak = nc.dram_tensor("rak", [N, D], BF16, kind="ExternalInput")
    i_rbk = nc.dram_tensor("rbk", [N, D], BF16, kind="ExternalInput")
    if with_proj_bias:
        i_pb = nc.dram_tensor("pb", [128, KC], F32, kind="ExternalInput")
    o_FT = nc.dram_tensor("FT", [C, N], F32, kind="ExternalOutput")

    from contextlib import ExitStack
    with tile.TileContext(nc) as tc:
        with ExitStack() as ctx:
            pp = ctx.enter_context(tc.tile_pool(name="persist", bufs=1))
            wpool = ctx.enter_context(tc.tile_pool(name="wpool", bufs=1))
            xload = ctx.enter_context(tc.tile_pool(name="xload", bufs=xlb))
            scratch = ctx.enter_context(tc.tile_pool(name="scratch", bufs=scb))
            natq = ctx.enter_context(tc.tile_pool(name="natq", bufs=nqb))
            natk = ctx.enter_context(tc.tile_pool(name="natk", bufs=nqb))
            sumsp = ctx.enter_context(tc.tile_pool(name="sumsp", bufs=4))
            ptp = ctx.enter_context(tc.tile_pool(name="ptp", bufs=ptb))
            rlp = ctx.enter_context(tc.tile_pool(name="rlp", bufs=2))
            rbp = ctx.enter_context(tc.tile_pool(name="rbp", bufs=2))
            outp = ctx.enter_context(tc.tile_pool(name="outp", bufs=4))
            # one PSUM pool, 4 tags x 2 banks (8 banks total), shared by all
            # phases: scA/scB + avA/avB
            psp = ctx.enter_context(tc.tile_pool(name="psp", bufs=1, space="PSUM"))

            # ---- persistent tensors ----
            # DMA emission order matters: the sync queue drains in order, so
            # interleave the first x chunks with the qkv weights (q first)
            # and push wp (phase-3-only) to the back.
            xm_t = {}

            def xm_prefetch(m):
                if m >= NCHUNK:
                    return
                xm = xload.tile([128, KC, 128], XDT, tag="xm", name=f"xm{m}")
                nc.sync.dma_start(out=xm[:], in_=i_xT[m])
                xm_t[m] = xm

            xm_prefetch(0)
            w_sb = {}
            # per-kc weight chunks: the m=0 qkv matmuls accumulate kc=0..7
            # in order, so they can chase the chunk DMAs instead of waiting
            # for the full 1MB tile
            for pf, (nm, src) in enumerate(
                    (("q", i_wq), ("k", i_wk), ("v", i_wv))):
                t = wpool.tile([128, KC, FL], XDT, tag=f"w{nm}")
                srcv = src[:].rearrange("(kc c) f -> c kc f", c=128)
                for kc in range(KC):
                    nc.sync.dma_start(out=t[:, kc, :], in_=srcv[:, kc, :])
                w_sb[nm] = t
                xm_prefetch(pf + 1)
            if with_qkv_bias:
                qkvb_sb = wpool.tile([1, 3 * FL], XDT, tag="qkvb")
                nc.sync.dma_start(out=qkvb_sb[:], in_=i_qkvb[:])
                ones1 = wpool.tile([1, 128], XDT, tag="ones1")
                nc.sync.dma_start(out=ones1[:], in_=i_ones1[:])
            if with_proj_bias:
                pb_sb = wpool.tile([128, KC], F32, tag="pb")
                nc.sync.dma_start(out=pb_sb[:], in_=i_pb[:])
            ident = wpool.tile([128, 128], BF16, tag="ident")
            make_identity(nc, ident[:])

            qT = pp.tile([128, 4, N], BF16, tag="qT")     # [f%128, fc, n]
            kT = pp.tile([128, 4, N], BF16, tag="kT")
            yT = pp.tile([128, 4, N], BF16, tag="yT")
            vball = pp.tile([128, NCHUNK, HL * 96], BF16, tag="vball")
            # per-head 96-col block: [1 | zeros(31) | v(64)].  Even head h
            # slices [96h+32, 96h+160): oT rows 0-63, sums row 64.  Odd head h
            # slices [96h-32, 96h+96): oT rows 64-127, sums row 32.  Unused
            # psum rows collect garbage and are never read.
            vview = vball[:].rearrange("p jc (h c) -> p jc h c", c=96)
            nc.vector.memset(vview[:, :, :, 0:32], 0.0)
            nc.vector.memset(vview[:, :, :, 0:1], 1.0)

            rope_sb = {}
            for nm2, srct in (("raq", i_raq), ("rbq", i_rbq),
                              ("rak", i_rak), ("rbk", i_rbk)):
                rt = wpool.tile([128, NCHUNK, D], BF16, tag=nm2)
                nc.sync.dma_start(out=rt[:], in_=srct[:].rearrange(
                    "(m p) d -> p m d", p=128))
                rope_sb[nm2] = rt

            wp_sb = wpool.tile([128, 4, C], BF16, tag="wp")
            nc.sync.dma_start(out=wp_sb[:], in_=i_wp[:].rearrange(
                "(kc c) o -> c kc o", c=128))

            # ================= Phase 1: qkv + rmsnorm + rope + transpose ====
            # Software-pipelined across n-chunks: S1 (qkv matmuls + psum
            # copies), S2 (rms stats + rope), S3 (PE transposes) are emitted
            # with a skew so each engine's FIFO interleaves iterations.
            loop_ctx = tc.For_i(0, bench_reps, 1) if bench_reps else None
            if loop_ctx is not None:
                ctx.enter_context(loop_ctx)

            nat_t = {}
            ro_t = {}
            qkv_tags = ("sc0", "sc1", "avA", "avB")

            def stage1(m):
                xm = xm_t.pop(m)
                xm_prefetch(m + 4)
                nat = {"q": natq.tile([128, FL], BF16, tag="qnat", name=f"qnat{m}"),
                       "k": natk.tile([128, FL], BF16, tag="knat", name=f"knat{m}")}
                nat_t[m] = nat
                for ti, nm in enumerate(("q", "k", "v")):
                    psum = psp.tile([128, FL], F32,
                                    tag=qkv_tags[(3 * m + ti) % 4],
                                    name=f"qkvps{m}_{ti}")
                    if with_qkv_bias:
                        nc.tensor.matmul(
                            psum[:], ones1[:],
                            qkvb_sb[:, ti * FL:(ti + 1) * FL],
                            start=True, stop=False)
                    for kc in range(KC):
                        nc.tensor.matmul(psum[:], xm[:, kc, :],
                                         w_sb[nm][:, kc, :],
                                         start=(kc == 0 and not with_qkv_bias),
                                         stop=(kc == KC - 1))
                    if nm == "v":
                        # straight into AV layout [n_p, jc=m, head, d]
                        nc.scalar.copy(
                            out=vview[:, m, :, 32:96],
                            in_=psum[:].rearrange("p (h d) -> p h d", h=HL))
                    else:
                        nc.scalar.copy(out=nat[nm][:], in_=psum[:])

            rstd_t = {}

            def stage2a(m):
                nat = nat_t[m]
                sums = sumsp.tile([128, 2 * HL], F32, tag="sums",
                                  name=f"sums{m}")
                for si, nm in enumerate(("q", "k")):
                    sq = scratch.tile([128, FL], BF16, tag="sxt",
                                      name=f"sq_{m}_{si}")
                    nc.gpsimd.tensor_mul(sq[:], nat[nm][:], nat[nm][:])
                    nc.vector.tensor_reduce(
                        sums[:, si * HL:(si + 1) * HL],
                        sq[:].rearrange("p (h d) -> p h d", h=HL),
                        axis=mybir.AxisListType.X, op=mybir.AluOpType.add)
                rstd_t[m] = sums

            def stage2b(m):
                nsl = slice(m * 128, (m + 1) * 128)
                nat = nat_t.pop(m)
                sums = rstd_t.pop(m)
                # rstd = sqrt(1/(ms + eps)).  Ln+Exp would thrash the ACT
                # table sets against phase 2's Exp; recip on DVE (fast
                # approx, ~18 bits — plenty) + Sqrt keeps one set per phase.
                msum = sumsp.tile([128, 2 * HL], F32, tag="lns", name=f"lns{m}")
                nc.vector.tensor_scalar(
                    msum[:], sums[:], 1.0 / D, EPS,
                    op0=mybir.AluOpType.mult, op1=mybir.AluOpType.add)
                rinv = sumsp.tile([128, 2 * HL], F32, tag="rinv",
                                  name=f"rinv{m}")
                nc.vector.reciprocal_approx_fast(rinv[:], msum[:])
                rstd = sumsp.tile([128, 2 * HL], BF16, tag="rstd",
                                  name=f"rstd{m}")
                nc.scalar.sqrt(rstd[:], rinv[:])
                for si, (nm, ra, rb) in enumerate(
                        (("q", "raq", "rbq"), ("k", "rak", "rbk"))):
                    rat = rope_sb[ra][:, m, :]
                    rbt = rope_sb[rb][:, m, :]

                    xv = nat[nm][:].rearrange("p (h d) -> p h d", h=HL)
                    rsview = bass.AP(
                        tensor=rstd.tensor, offset=rstd[:].offset + si * HL,
                        ap=[rstd[:].ap[0], [1, HL], [0, D]])
                    xn = scratch.tile([128, FL], BF16, tag="sxt",
                                      name=f"xn_{m}_{si}")
                    xnv = xn[:].rearrange("p (h d) -> p h d", h=HL)
                    nc.vector.tensor_mul(xnv, xv, rsview)

                    rav = bass.AP(tensor=rat.tensor, offset=rat.offset,
                                  ap=[rat.ap[0], [0, HL], [1, D]])
                    t1 = scratch.tile([128, FL], BF16, tag="sxt",
                                      name=f"t1_{m}_{si}")
                    t1v = t1[:].rearrange("p (h d) -> p h d", h=HL)
                    nc.vector.tensor_mul(t1v, xnv, rav)

                    ro = scratch.tile([128, FL], BF16, tag="ro", bufs=4,
                                      name=f"ro_{m}_{si}")
                    rov = ro[:].rearrange("p (h d) -> p h d", h=HL)
                    rb_lo = bass.AP(tensor=rbt.tensor, offset=rbt.offset,
                                    ap=[rbt.ap[0], [0, HL], [1, 32]])
                    rb_hi = bass.AP(tensor=rbt.tensor, offset=rbt.offset + 32,
                                    ap=[rbt.ap[0], [0, HL], [1, 32]])
                    nc.vector.tensor_mul(rov[:, :, 0:32], xnv[:, :, 32:64], rb_lo)
                    nc.vector.tensor_mul(rov[:, :, 32:64], xnv[:, :, 0:32], rb_hi)
                    nc.gpsimd.tensor_add(ro[:], ro[:], t1[:])
                    ro_t[(m, si)] = ro

            def stage3(m):
                nsl = slice(m * 128, (m + 1) * 128)
                for si, dst in ((0, qT), (1, kT)):
                    ro = ro_t.pop((m, si))
                    for fc in range(4):
                        tp = psp.tile([128, 128], BF16, tag=f"av{'AB'[fc % 2]}",
                                      name=f"trps{m}_{si}_{fc}")
                        nc.tensor.transpose(tp[:], ro[:, fc * 128:(fc + 1) * 128],
                                            ident[:])
                        if fc % 2:
                            nc.scalar.copy(out=dst[:, fc, nsl], in_=tp[:])
                        else:
                            nc.vector.tensor_copy(out=dst[:, fc, nsl], in_=tp[:])

            for t in range(NCHUNK):
                if t >= 3:
                    stage3(t - 3)
                if 2 <= t < NCHUNK + 2:
                    stage2b(t - 2)
                if 1 <= t < NCHUNK + 1:
                    stage2a(t - 1)
                if t < NCHUNK:
                    stage1(t)
            # phase-1 drain (stage3 of the last 3 chunks) is emitted below,
            # interleaved with the first phase-2 units

            # ================= Phase 2: attention, flat pipeline ============
            # All 8 (fc, ih) blocks flattened into one 128-step pipeline:
            # step t emits scores+exps of unit t and the AV matmuls of unit
            # t-SKEW (which may belong to the previous block).  Block
            # boundaries disappear from the PE FIFO: the next block's score
            # groups always sit between a block's last AVs, so the PE never
            # drains while ACT catches up, and the psum readout of block b
            # overlaps block b+1's scores.
            PHASES = int(os.environ.get("KERNEL_PHASES", "3"))
            NOSM = bool(os.environ.get("KERNEL_NOSM"))
            P2_SKEW = 2
            blocks = [(fc, ih) for fc in range(4) for ih in range(IH)]
            av_t = {}
            pt_t = {}

            def scores_exp(b, jc):
                fc, ih = blocks[b]
                jsl = slice(jc * 128, (jc + 1) * 128)
                for half in range(2):
                    po = 64 * half
                    sc = psp.tile([128, 1024], F32, tag=f"sc{half}",
                                  name=f"sc{b}_{jc}_{half}")
                    lhs = kT[po:po + 64, fc, jsl]
                    for i2 in range(2):
                        nc.tensor.matmul(
                            sc[:, i2 * 512:(i2 + 1) * 512], lhs,
                            qT[po:po + 64, fc,
                               ih * 1024 + i2 * 512:
                               ih * 1024 + (i2 + 1) * 512],
                            start=True, stop=True)
                    if NOSM:
                        continue
                    pt = ptp.tile([128, 1024], BF16, tag=f"pt{half}",
                                  name=f"pt{b}_{jc}_{half}")
                    nc.scalar.activation(pt[:], sc[:], AF.Exp,
                                         scale=float(SCALE))
                    pt_t[(b, jc, half)] = pt

            def do_av(b, jc):
                fc, ih = blocks[b]
                if jc == 0:
                    av_t[b] = {
                        0: psp.tile([128, 1024], F32, tag="avA",
                                    name=f"avA{b}"),
                        1: psp.tile([128, 1024], F32, tag="avB",
                                    name=f"avB{b}")}
                av = av_t[b]
                for half in range(2):
                    h = 2 * fc + half
                    vs = 96 * h + 32 if half == 0 else 96 * h - 32
                    pt = pt_t.pop((b, jc, half))
                    for i2 in range(2):
                        nc.tensor.matmul(
                            av[half][:, i2 * 512:(i2 + 1) * 512],
                            vball[:, jc, vs:vs + 128],
                            pt[:, i2 * 512:(i2 + 1) * 512],
                            start=(jc == 0), stop=(jc == NCHUNK - 1))

            def normalize(b):
                # yT[f, n] = av_oT[f, n] * (1 / av_sums[n]).  even head: oT
                # rows 0-63, sums row 64; odd head: oT rows 64-127, sums row
                # 32 (lane-aligned by vball layout).  readout on DVE (frees
                # the banks); recip (fast DVE approx, staged to partition 0
                # --- nonzero input base mangles on HW) + gpsimd
                # partition-broadcast; mul back on DVE (gpsimd would thrash
                # Q7 libraries against partition_broadcast).
                fc, ih = blocks[b]
                isl = slice(ih * 1024, (ih + 1) * 1024)
                av = av_t.pop(b)
                if os.environ.get("KERNEL_NONORM"):
                    return
                for half in range(2):
                    po = 64 * half
                    srow = 64 if half == 0 else 32
                    avs = rlp.tile([128, 1024], F32, tag="avs", bufs=avb,
                                   name=f"avs{b}_{half}")
                    r0 = rlp.tile([1, 1024], F32, tag="r0",
                                  name=f"r0{b}_{half}")
                    if half == 0:
                        nc.vector.tensor_copy(out=avs[0:65, :],
                                              in_=av[half][0:65, :])
                        nc.vector.tensor_copy(out=r0[:], in_=avs[64:65, :])
                    else:
                        nc.vector.tensor_copy(out=avs[64:128, :],
                                              in_=av[half][64:128, :])
                        nc.vector.tensor_copy(out=r0[:],
                                              in_=av[half][srow:srow + 1, :])
                    r_l = rlp.tile([1, 1024], F32, tag="rl",
                                   name=f"rl{b}_{half}")
                    nc.vector.reciprocal_approx_fast(r_l[:], r0[:])
                    rbc = rbp.tile([128, 1024], F32, tag="rbc",
                                   name=f"rbc{b}_{half}")
                    nc.gpsimd.partition_broadcast(rbc[:], r_l[:],
                                                  channels=128)
                    nc.vector.tensor_mul(yT[po:po + 64, fc, isl],
                                         avs[po:po + 64, :],
                                         rbc[po:po + 64, :])

            NBLK = len(blocks) if PHASES >= 2 else 0
            P2_HEAD = 3 if NBLK else 0
            # interleave the phase-1 drain with the first phase-2 units: the
            # PE chews block-0 scores while the DVE finishes the last rope
            # chains instead of idling (which re-throttled the clock gate
            # right at phase-2 entry)
            for u in range(3):
                if u < P2_HEAD:
                    scores_exp(*divmod(u, NCHUNK))
                t = NCHUNK + u
                if 2 <= t < NCHUNK + 2:
                    stage2b(t - 2)
                if t < NCHUNK + 1:
                    stage2a(t - 1)
                stage3(t - 3)
                # do_av(0) must come after the last transpose: it pins the
                # avA/avB psum slots for the whole block-0 accumulation
                if u == 2 and P2_HEAD and not NOSM:
                    do_av(*divmod(0, NCHUNK))
            for t in range(P2_HEAD, NBLK * NCHUNK + (P2_SKEW if NBLK else 0)):
                if t < NBLK * NCHUNK:
                    b, jc = divmod(t, NCHUNK)
                    scores_exp(b, jc)
                if t >= P2_SKEW and not NOSM:
                    b2, jc2 = divmod(t - P2_SKEW, NCHUNK)
                    do_av(b2, jc2)
                    if jc2 == NCHUNK - 1:
                        normalize(b2)

            # ================= Phase 3: output projection ===================
            if PHASES < 3:
                nc.sync.dma_start(out=o_FT[:].rearrange('c n -> (c n)')[0:C * N // (2 if qkv_bf16 else 1)].bitcast(XDT), in_=i_xT[:].rearrange('m c kc n -> (m c kc n)'))
            for cc in range(KC if PHASES >= 3 else 0):
                for nt in range(4):
                    fp = psp.tile([128, 512], F32,
                                  tag=("sc0", "sc1", "avA", "avB")[(cc * 4 + nt) % 4],
                                  name=f"fp{cc}_{nt}")
                    for kc in range(4):
                        nc.tensor.matmul(
                            fp[:], wp_sb[:, kc, cc * 128:(cc + 1) * 128],
                            yT[:, kc, nt * 512:(nt + 1) * 512],
                            start=(kc == 0), stop=(kc == 3))
                    so = outp.tile([128, 512], F32, tag="so")
                    if with_proj_bias:
                        nc.scalar.activation(so[:], fp[:], AF.Identity,
                                             bias=pb_sb[:, cc:cc + 1])
                    elif (cc * 4 + nt) % 2:
                        nc.scalar.copy(out=so[:], in_=fp[:])
                    else:
                        nc.vector.tensor_copy(out=so[:], in_=fp[:])
                    nc.sync.dma_start(
                        out=o_FT[cc * 128:(cc + 1) * 128,
                                 nt * 512:(nt + 1) * 512],
                        in_=so[:])

    nc.compile()
    return nc


def _host_prep(x, qkv_w, qkv_b, proj_w, proj_b, qn_w, kn_w, rope_cos, rope_sin,
               qkv_bf16=True):
    xdt = ml_dtypes.bfloat16 if qkv_bf16 else np.float32
    """Build the 8 per-core input maps."""
    x = np.asarray(x, dtype=np.float32)
    qkv_w = np.asarray(qkv_w, dtype=np.float32)
    qkv_b = np.asarray(qkv_b, dtype=np.float32)
    proj_w = np.asarray(proj_w, dtype=np.float32)
    proj_b = np.asarray(proj_b, dtype=np.float32)
    qn_w = np.asarray(qn_w, dtype=np.float32)
    kn_w = np.asarray(kn_w, dtype=np.float32)
    rope_cos = np.asarray(rope_cos, dtype=np.float32)
    rope_sin = np.asarray(rope_sin, dtype=np.float32)

    # rope tables with rmsnorm weight folded in:
    # out[0:32]  = xh[0:32]*(w*cos)[0:32]  + xh[32:64]*(-w2*sin[0:32])
    # out[32:64] = xh[32:64]*(w*cos)[32:64] + xh[0:32]*( w1*sin[32:64])
    def tables(w):
        a = rope_cos * w[None, :]
        b = np.empty_like(rope_sin)
        b[:, 0:32] = -rope_sin[:, 0:32] * w[None, 32:64]
        b[:, 32:64] = rope_sin[:, 32:64] * w[None, 0:32]
        return (np.ascontiguousarray(a).astype(ml_dtypes.bfloat16),
                np.ascontiguousarray(b).astype(ml_dtypes.bfloat16))

    raq, rbq = tables(qn_w)
    rak, rbk = tables(kn_w)
    with_qkv_bias = bool(np.any(qkv_b))
    with_proj_bias = bool(np.any(proj_b))
    ones1 = np.ones((1, 128), dtype=np.float32)
    pb = np.ascontiguousarray(proj_b.reshape(KC, 128).T)

    # shared tensors are computed once per batch / head-half, not per core
    xT = {}
    for b in range(4):
        xT[b] = np.ascontiguousarray(
            x[b].T.reshape(KC, 128, NCHUNK, 128).transpose(2, 1, 0, 3)
        ).astype(xdt)
    wmaps = {}
    for hh in range(2):
        fsl = slice(hh * FL, hh * FL + FL)
        wmaps[hh] = {
            "wq": np.ascontiguousarray(qkv_w[fsl, :].T).astype(xdt),
            "wk": np.ascontiguousarray(qkv_w[C:][fsl, :].T).astype(xdt),
            "wv": np.ascontiguousarray(qkv_w[2 * C:][fsl, :].T).astype(xdt),
            "wp": np.ascontiguousarray(proj_w[:, fsl].T).astype(ml_dtypes.bfloat16),
        }
        if with_qkv_bias:
            wmaps[hh]["qkvb"] = np.concatenate(
                [qkv_b[fsl], qkv_b[C:][fsl], qkv_b[2 * C:][fsl]]
            ).reshape(1, 3 * FL).astype(xdt)

    in_maps = []
    for ci in range(8):
        b, hh = divmod(ci, 2)
        m = {
            "xT": xT[b],
            "raq": raq, "rbq": rbq, "rak": rak, "rbk": rbk,
            **wmaps[hh],
        }
        if with_qkv_bias:
            m["ones1"] = ones1.astype(xdt)
        if with_proj_bias:
            m["pb"] = pb
        in_maps.append(m)
    return in_maps, with_qkv_bias, with_proj_bias


def kernel(x, qkv_w, qkv_b, proj_w, proj_b, qn_w, kn_w, rope_cos, rope_sin,
           _trace=False):
    global _PROGRAM
    in_maps, wqb, wpb = _host_prep(x, qkv_w, qkv_b, proj_w, proj_b, qn_w, kn_w,
                                   rope_cos, rope_sin)
    if _PROGRAM is None or _PROGRAM[0] != (wqb, wpb):
        _PROGRAM = ((wqb, wpb), _build_program(wqb, wpb))
    nc = _PROGRAM[1]
    kwargs = {}
    if _trace:
        kwargs = dict(trace=True, trace_cores=[0])
    res = run_bass_kernel_spmd(nc, in_maps, core_ids=list(range(8)), **kwargs)
    if _trace:
        kernel.last_exec_ns = res.exec_time_ns
        kernel.last_results = res
    out = np.empty((B, N, C), dtype=np.float32)
    for b in range(B):
        ft = res.results[2 * b]["FT"] + res.results[2 * b + 1]["FT"]
        out[b] = ft.T
    return out

